# revision 8
# baseline (speedup 1.0000x reference)
"""AFTLocal kernel for 8 TRN2 NeuronCores.

Math: the reference's numerator/denominator = (dw*exp_k*v)/(dw*exp_k) = v
elementwise (all factors finite and > 0), so the module reduces exactly to

    out = (sigmoid(X @ Wq + bq) * (X @ Wv + bv)) @ Wo + bo

Sharding: data-parallel over batch. Each of the 8 cores processes 8 batches
(1024 tokens) with replicated weights; no collectives.

Per-core pipeline (all bf16 matmuls, f32 PSUM accumulate):
  - load Wq/Wv/Wo f32 -> convert bf16 (resident in SBUF)
  - load X f32 -> convert bf16 -> PE-transpose into XT[k] (Dm on partitions)
  - stage 1: QT/VT tiles = Wq/Wv chunk.T @ XT  (PSUM), fused epilogue
             HT = sigmoid(QT + bq) * (VT + bv)  (bf16, resident)
  - stage 2: out tile = HT.T-chunks @ Wo + bo -> f32 -> DMA out
"""

import numpy as np

B, S, DM, DI = 64, 128, 1024, 1024
NCORES = 8
BL = B // NCORES          # batches per core
T = BL * S                # tokens per core = 1024
P = 128                   # partitions
KC = DM // P              # 8 contraction chunks
NT = T // P               # 8 token tiles of 128
NF = 512                  # matmul moving free dim (one PSUM bank of f32)
TN = T // NF              # 2 token blocks of 512
DN = DM // NF             # 2 output column blocks of 512

_CACHE = {}

# walrus in this container only supports 1 sync-wait per instruction for
# several ISA structs; Tile emits up to one wait per logical proc. Split
# excess waits into a chain of single-wait NoOps on the same engine
# (same-engine program order makes this equivalent).
def _split_waits(nc):
    from concourse import mybir

    for f in nc.m.functions:
        for b in f.blocks:
            new = []
            changed = False
            for inst in b.instructions:
                si = getattr(inst, "sync_info", None)
                limit = 1
                if si is not None and len(si.on_wait) > limit:
                    waits = list(si.on_wait)
                    extra, keep = waits[:-limit], waits[-limit:]
                    for i, w in enumerate(extra):
                        new.append(mybir.InstNoOp(
                            name=f"{inst.name}-wsplit{i}", ins=[], outs=[],
                            engine=inst.engine,
                            sync_info=mybir.SyncInfo(on_wait=[w], on_update=[]),
                        ))
                    inst.sync_info = mybir.SyncInfo(
                        on_wait=keep, on_update=list(si.on_update))
                    changed = True
                new.append(inst)
            if changed:
                b.instructions = new


def _build():
    import concourse.bass as bass
    import concourse.tile as tile
    from concourse import mybir
    from concourse.masks import make_identity
    from contextlib import ExitStack

    f32 = mybir.dt.float32
    bf16 = mybir.dt.bfloat16
    Act = mybir.ActivationFunctionType
    Alu = mybir.AluOpType

    nc = bass.Bass("TRN2")
    x_d = nc.dram_tensor("x", [T, DM], f32, kind="ExternalInput")
    wq_d = nc.dram_tensor("wq", [DM, DI], f32, kind="ExternalInput")
    wv_d = nc.dram_tensor("wv", [DM, DI], f32, kind="ExternalInput")
    wo_d = nc.dram_tensor("wo", [DI, DM], f32, kind="ExternalInput")
    bq_d = nc.dram_tensor("bq", [1, DI], f32, kind="ExternalInput")
    bv_d = nc.dram_tensor("bv", [1, DI], f32, kind="ExternalInput")
    bo_d = nc.dram_tensor("bo", [1, DM], f32, kind="ExternalInput")
    out_d = nc.dram_tensor("out", [T, DM], f32, kind="ExternalOutput")

    with ExitStack() as ctx:
        tc = ctx.enter_context(tile.TileContext(nc))
        consts = ctx.enter_context(tc.tile_pool(name="consts", bufs=1))
        wpool = ctx.enter_context(tc.tile_pool(name="weights", bufs=1))
        xtp = ctx.enter_context(tc.tile_pool(name="xt", bufs=1))
        htp = ctx.enter_context(tc.tile_pool(name="ht", bufs=1))
        stage = ctx.enter_context(tc.tile_pool(name="stage", bufs=3))
        sigp = ctx.enter_context(tc.tile_pool(name="sig", bufs=3))
        opool = ctx.enter_context(tc.tile_pool(name="opool", bufs=3))
        psum = ctx.enter_context(tc.tile_pool(name="psum", bufs=2, space="PSUM"))

        # ---- constants ----
        ident = consts.tile([P, P], bf16)
        make_identity(nc, ident)
        ones = consts.tile([1, NF], bf16)
        nc.vector.memset(ones, 1.0)

        # bias rows (bf16 for matmul use, cast during SWDGE DMA)
        bq_row = consts.tile([1, DI], bf16)
        bv_row = consts.tile([1, DI], bf16)
        bo_row = consts.tile([1, DM], bf16)
        nc.gpsimd.dma_start(out=bq_row, in_=bq_d[:, :])
        nc.gpsimd.dma_start(out=bv_row, in_=bv_d[:, :])
        nc.gpsimd.dma_start(out=bo_row, in_=bo_d[:, :])

        # per-partition bias columns via K=1 matmuls (row -> column transpose)
        bq_pp = consts.tile([P, KC], f32)
        bv_pp = consts.tile([P, KC], f32)
        for di in range(KC):
            pc = psum.tile([P, 1], f32, tag="pst")
            nc.tensor.matmul(pc, bq_row[:, di * P:(di + 1) * P], ones[:, :1],
                             start=True, stop=True)
            nc.vector.tensor_copy(bq_pp[:, di:di + 1], pc)
            pc2 = psum.tile([P, 1], f32, tag="pst")
            nc.tensor.matmul(pc2, bv_row[:, di * P:(di + 1) * P], ones[:, :1],
                             start=True, stop=True)
            nc.vector.tensor_copy(bv_pp[:, di:di + 1], pc2)

        # bo broadcast to all partitions (added into the output epilogue):
        # DMA with a stride-0 partition AP replicates the DRAM row 128x.
        bo_bc = consts.tile([P, DM], f32)
        bo_ap = bo_d[:, :]
        bo_bcast_src = bass.AP(tensor=bo_ap.tensor, offset=bo_ap.offset,
                               ap=[[0, P]] + list(bo_ap.ap)[1:])
        nc.gpsimd.dma_start(out=bo_bc, in_=bo_bcast_src)

        # ---- weights + X: load with f32->bf16 cast during DMA (SWDGE) ----
        wq_bf = [wpool.tile([P, DI], bf16, tag=f"wq{k}", name=f"wq_bf{k}") for k in range(KC)]
        wv_bf = [wpool.tile([P, DI], bf16, tag=f"wv{k}", name=f"wv_bf{k}") for k in range(KC)]
        wo_bf = [wpool.tile([P, DM], bf16, tag=f"wo{k}", name=f"wo_bf{k}") for k in range(KC)]

        # X tiles: cast-DMA then PE-transpose into XT
        xt = [xtp.tile([P, T], bf16, tag=f"xt{k}", name=f"xt{k}") for k in range(KC)]
        for t in range(NT):
            xbf = stage.tile([P, DM], bf16, tag="xbf")
            nc.gpsimd.dma_start(out=xbf, in_=x_d[t * P:(t + 1) * P, :])
            nc.gpsimd.dma_start(out=wq_bf[t], in_=wq_d[t * P:(t + 1) * P, :])
            nc.gpsimd.dma_start(out=wv_bf[t], in_=wv_d[t * P:(t + 1) * P, :])
            for k in range(KC):
                pst = psum.tile([P, P], bf16, tag="pst")
                nc.tensor.transpose(pst, xbf[:, k * P:(k + 1) * P], ident)
                nc.vector.tensor_copy(xt[k][:, t * P:(t + 1) * P], pst)
        for k in range(KC):
            nc.gpsimd.dma_start(out=wo_bf[k], in_=wo_d[k * P:(k + 1) * P, :])

        # ---- stage 1: HT[di] = sigmoid(QT + bq) * (VT + bv) ----
        ht = [htp.tile([P, T], bf16, tag=f"ht{k}", name=f"ht{k}") for k in range(KC)]
        for di in range(KC):
            for tn in range(TN):
                ts = slice(tn * NF, (tn + 1) * NF)
                ps_q = psum.tile([P, NF], f32, tag="psq")
                ps_v = psum.tile([P, NF], f32, tag="psv")
                for k in range(KC):
                    nc.tensor.matmul(ps_q, wq_bf[k][:, di * P:(di + 1) * P],
                                     xt[k][:, ts], start=(k == 0), stop=(k == KC - 1))
                for k in range(KC):
                    nc.tensor.matmul(ps_v, wv_bf[k][:, di * P:(di + 1) * P],
                                     xt[k][:, ts], start=(k == 0), stop=(k == KC - 1))
                sig = sigp.tile([P, NF], bf16, tag="sig")
                nc.scalar.activation(sig, ps_q, Act.Sigmoid,
                                     bias=bq_pp[:, di:di + 1])
                nc.vector.scalar_tensor_tensor(
                    out=ht[di][:, ts], in0=ps_v, scalar=bv_pp[:, di:di + 1],
                    in1=sig, op0=Alu.add, op1=Alu.mult)

        # ---- stage 2: out = HT.T @ Wo + bo ----
        for t in range(NT):
            rs = slice(t * P, (t + 1) * P)
            for n in range(DN):
                cs = slice(n * NF, (n + 1) * NF)
                ps_o = psum.tile([P, NF], f32, tag="pso")
                for k in range(KC):
                    nc.tensor.matmul(ps_o, ht[k][:, rs], wo_bf[k][:, cs],
                                     start=(k == 0), stop=(k == KC - 1))
                ob = opool.tile([P, NF], f32, tag="ob")
                nc.vector.tensor_tensor(out=ob, in0=ps_o, in1=bo_bc[:, cs],
                                        op=Alu.add)
                nc.sync.dma_start(out=out_d[rs, cs], in_=ob)

    _split_waits(nc)
    return nc


def _get_nc():
    if "nc" not in _CACHE:
        _CACHE["nc"] = _build()
    return _CACHE["nc"]


def run(inputs, trace=False):
    """inputs: dict with setup_inputs() keys (numpy). Returns (out, exec_time_ns)."""
    from concourse import bass_utils

    nc = _get_nc()
    x = np.ascontiguousarray(np.asarray(inputs["embeddings"], dtype=np.float32)
                             ).reshape(B * S, DM)
    wq = np.ascontiguousarray(np.asarray(inputs["Wq"], dtype=np.float32))
    wv = np.ascontiguousarray(np.asarray(inputs["Wv"], dtype=np.float32))
    wo = np.ascontiguousarray(np.asarray(inputs["Wo"], dtype=np.float32))
    bq = np.asarray(inputs["bq"], dtype=np.float32).reshape(1, DI)
    bv = np.asarray(inputs["bv"], dtype=np.float32).reshape(1, DI)
    bo = np.asarray(inputs["bo"], dtype=np.float32).reshape(1, DM)

    in_maps = []
    for c in range(NCORES):
        in_maps.append({
            "x": x[c * T:(c + 1) * T],
            "wq": wq, "wv": wv, "wo": wo,
            "bq": bq, "bv": bv, "bo": bo,
        })
    res = bass_utils.run_bass_kernel_spmd(
        nc, in_maps, core_ids=list(range(NCORES)), trace=trace)
    out = np.concatenate([r["out"] for r in res.results], axis=0)
    return out.reshape(B, S, DM).astype(np.float32), res.exec_time_ns


def kernel(**inputs):
    out, _ = run(inputs, trace=False)
    return out
